# revision 1
# baseline (speedup 1.0000x reference)
"""JambaMoE Trainium2 kernel: expert-parallel MoE with host-side token dispatch.

Strategy (sharding_hint: expert parallelism):
  - 8 experts, 8 cores: core e owns expert e's weights.
  - Router (tiny: [T,2048]@[2048,8]) + top-2 + softmax run on host during
    input sharding; tokens are gathered per expert, padded to a common
    capacity C, and dispatched to the owning core.
  - Each core runs a SiLU-gated MLP (gate/up [4096,2048], down [2048,4096])
    over its C tokens in bf16 (fp32 PSUM accumulation), all data pre-packed
    host-side into DMA-friendly SBUF layouts (contraction dim on partitions).
  - Host scatter-adds the combine-weighted expert outputs back to [B,S,D].

Device kernel is raw Bass (explicit semaphores): this container's walrus
rejects Tile-generated multi-wait instructions ("Too many sync wait
commands"), so all cross-engine sync uses standalone single-sem waits with
cumulative thresholds.
"""

import numpy as np
import ml_dtypes

import concourse.bass as bass
import concourse.mybir as mybir
from concourse.bass_utils import run_bass_kernel_spmd

B, S, D, E, I, TOP_K = 2, 4096, 2048, 8, 4096, 2
N_CORES = 8
TN = 512          # token tile (free dim per matmul)
DC = D // 128     # 16 contraction chunks for gate/up
IB = I // 128     # 32 intermediate blocks
DB = D // 128     # 16 output-dim blocks
BF16 = mybir.dt.bfloat16
FP32 = mybir.dt.float32


def build_kernel(C: int):
    """Raw-Bass SPMD kernel for one expert shard: y = (silu(x@Wg.T)*(x@Wu.T))@W2.T

    Inputs (packed, see kernel()):
      x:  [128, DC, C]   bf16  (x[t, dc*128+dp] at [dp, dc, t])
      wg: [IB, 128, DC, 128] bf16  (packed gate tiles, contraction on partitions)
      wu: [IB, 128, DC, 128] bf16
      w2: [DB, 128, IB, 128] bf16
    Output:
      y:  [DB, 128, C] fp32  (y[t, db*128+dp] at [db, dp, t])
    """
    TC = C // TN
    nslot = 3 if C <= 2560 else 2

    nc = bass.Bass()
    x_ext = nc.dram_tensor("x", [128, DC, C], BF16, kind="ExternalInput")
    wg_ext = nc.dram_tensor("wg", [IB, 128, DC, 128], BF16, kind="ExternalInput")
    wu_ext = nc.dram_tensor("wu", [IB, 128, DC, 128], BF16, kind="ExternalInput")
    w2_ext = nc.dram_tensor("w2", [DB, 128, IB, 128], BF16, kind="ExternalInput")
    y_ext = nc.dram_tensor("y", [DB, 128, C], FP32, kind="ExternalOutput")

    NTMP = 4   # silu temp buffers
    NY = 3     # y staging buffers

    with (
        nc.sbuf_tensor([128, DC, C], BF16) as x_sb,
        nc.sbuf_tensor([128, nslot, DC, 128], BF16) as wg_sb,
        nc.sbuf_tensor([128, nslot, DC, 128], BF16) as wu_sb,
        nc.sbuf_tensor([128, nslot, IB, 128], BF16) as w2_sb,
        nc.sbuf_tensor([128, IB, TN], BF16) as a_sb,
        nc.sbuf_tensor([128, NTMP, TN], FP32) as tmp_sb,
        nc.sbuf_tensor([128, NY, TN], FP32) as y_sb,
        nc.psum_tensor([128, 2, TN], FP32) as g_ps,
        nc.psum_tensor([128, 2, TN], FP32) as u_ps,
        nc.psum_tensor([128, 4, TN], FP32) as y_ps,
        nc.semaphore() as dma_in,   # sync-engine input DMAs (inc 16 each)
        nc.semaphore() as pe_a,     # PE group completions (inc 1)
        nc.semaphore() as act_s,    # ACT silu completions
        nc.semaphore() as dve_s,    # DVE mul/copy completions
        nc.semaphore() as out_s,    # gpsimd output DMAs (inc 16)
        nc.Block() as block,
    ):
        # ---- emit-time bookkeeping (python ints; programs are fully static)
        # DMA issue order on sync engine determines cumulative dma_in counts.
        dma_count = 0
        w_ready = {}     # ("g"|"u", tc, ib) or ("2", tc, db) -> dma_in threshold
        # PE program order: per tc: [g(ib), u(ib)] * IB, then y(db) * DB
        pe_count = 0
        g_end, u_end, y_end = {}, {}, {}
        for tc in range(TC):
            for ib in range(IB):
                pe_count += 1; g_end[(tc, ib)] = pe_count
                pe_count += 1; u_end[(tc, ib)] = pe_count
            for db in range(DB):
                pe_count += 1; y_end[(tc, db)] = pe_count
        # ACT order: silu per (tc, ib)
        silu_end = {}
        cnt = 0
        for tc in range(TC):
            for ib in range(IB):
                cnt += 1; silu_end[(tc, ib)] = cnt
        # DVE order: per tc: mul(ib)*IB then ycopy(db)*DB
        mul_end, ycopy_end = {}, {}
        cnt = 0
        for tc in range(TC):
            for ib in range(IB):
                cnt += 1; mul_end[(tc, ib)] = cnt
            for db in range(DB):
                cnt += 1; ycopy_end[(tc, db)] = cnt
        # output store order on gpsimd
        store_end = {}
        cnt = 0
        for tc in range(TC):
            for db in range(DB):
                cnt += 16; store_end[(tc, db)] = cnt

        @block.sync
        def _(sync):
            nonlocal dma_count
            dma_count = 0
            for dc in range(DC):
                sync.dma_start(x_sb[:, dc, :], x_ext[:, dc, :]).then_inc(dma_in, 16)
                dma_count += 16
            # slot -> pe_a threshold that frees it
            slot_free = {}
            for tc in range(TC):
                for ib in range(IB):
                    s = ib % nslot
                    for kind, sb, ext in (("g", wg_sb, wg_ext), ("u", wu_sb, wu_ext)):
                        key = (kind, s)
                        if key in slot_free:
                            sync.wait_ge(pe_a, slot_free[key])
                        sync.dma_start(sb[:, s], ext[ib]).then_inc(dma_in, 16)
                        dma_count += 16
                        w_ready[(kind, tc, ib)] = dma_count
                        slot_free[key] = (g_end if kind == "g" else u_end)[(tc, ib)]
                for db in range(DB):
                    s = db % nslot
                    key = ("2", s)
                    if key in slot_free:
                        sync.wait_ge(pe_a, slot_free[key])
                    sync.dma_start(w2_sb[:, s], w2_ext[db]).then_inc(dma_in, 16)
                    dma_count += 16
                    w_ready[("2", tc, db)] = dma_count
                    slot_free[key] = y_end[(tc, db)]

        @block.tensor
        def _(tensor):
            first = True
            for tc in range(TC):
                t0 = tc * TN
                for ib in range(IB):
                    s = ib % nslot
                    gb, ub = ib % 2, ib % 2
                    tensor.wait_ge(dma_in, w_ready[("u", tc, ib)])
                    if first:
                        first = False  # x loads precede all weight loads
                    # psum bank WAR: consumers of (tc, ib-2) done?
                    if (tc, ib - 2) in mul_end:
                        tensor.wait_ge(dve_s, mul_end[(tc, ib - 2)])
                    elif tc > 0 and ib < 2:
                        tensor.wait_ge(dve_s, mul_end[(tc - 1, IB - 2 + ib)])
                    for dc in range(DC):
                        mm = tensor.matmul(
                            g_ps[:, gb, :], wg_sb[:, s, dc, :],
                            x_sb[:, dc, t0:t0 + TN],
                            start=(dc == 0), stop=(dc == DC - 1),
                        )
                    mm.then_inc(pe_a, 1)
                    for dc in range(DC):
                        mm = tensor.matmul(
                            u_ps[:, ub, :], wu_sb[:, s, dc, :],
                            x_sb[:, dc, t0:t0 + TN],
                            start=(dc == 0), stop=(dc == DC - 1),
                        )
                    mm.then_inc(pe_a, 1)
                for db in range(DB):
                    s = db % nslot
                    yb = db % 4
                    tensor.wait_ge(dma_in, w_ready[("2", tc, db)])
                    # need all 32 muls of this tc; plus y bank free (copy db-4)
                    need = mul_end[(tc, IB - 1)]
                    if (tc, db - 4) in ycopy_end:
                        need = max(need, ycopy_end[(tc, db - 4)])
                    elif tc > 0:
                        need = max(need, ycopy_end[(tc - 1, DB - 4 + db)])
                    tensor.wait_ge(dve_s, need)
                    for ic in range(IB):
                        mm = tensor.matmul(
                            y_ps[:, yb, :], w2_sb[:, s, ic, :], a_sb[:, ic, :],
                            start=(ic == 0), stop=(ic == IB - 1),
                        )
                    mm.then_inc(pe_a, 1)

        @block.scalar
        def _(scalar):
            for tc in range(TC):
                for ib in range(IB):
                    ts = ib % NTMP
                    scalar.wait_ge(pe_a, g_end[(tc, ib)])
                    if (tc, ib - NTMP) in mul_end:
                        scalar.wait_ge(dve_s, mul_end[(tc, ib - NTMP)])
                    elif tc > 0 and ib < NTMP:
                        scalar.wait_ge(dve_s, mul_end[(tc - 1, IB - NTMP + ib)])
                    scalar.activation(
                        tmp_sb[:, ts, :], g_ps[:, ib % 2, :],
                        mybir.ActivationFunctionType.Silu,
                    ).then_inc(act_s, 1)

        @block.vector
        def _(vector):
            for tc in range(TC):
                for ib in range(IB):
                    ts = ib % NTMP
                    vector.wait_ge(act_s, silu_end[(tc, ib)])
                    vector.wait_ge(pe_a, u_end[(tc, ib)])
                    vector.tensor_mul(
                        a_sb[:, ib, :], tmp_sb[:, ts, :], u_ps[:, ib % 2, :]
                    ).then_inc(dve_s, 1)
                for db in range(DB):
                    ys = db % NY
                    vector.wait_ge(pe_a, y_end[(tc, db)])
                    if (tc, db - NY) in store_end:
                        vector.wait_ge(out_s, store_end[(tc, db - NY)])
                    elif tc > 0 and db < NY:
                        vector.wait_ge(out_s, store_end[(tc - 1, DB - NY + db)])
                    vector.tensor_copy(
                        y_sb[:, ys, :], y_ps[:, db % 4, :]
                    ).then_inc(dve_s, 1)

        @block.gpsimd
        def _(gpsimd):
            for tc in range(TC):
                t0 = tc * TN
                for db in range(DB):
                    ys = db % NY
                    gpsimd.wait_ge(dve_s, ycopy_end[(tc, db)])
                    gpsimd.dma_start(
                        y_ext[db, :, t0:t0 + TN], y_sb[:, ys, :]
                    ).then_inc(out_s, 16)
            gpsimd.wait_ge(out_s, 16 * DB * TC)

    return nc


def _route_host(h_flat, router_weight):
    """Replicate the reference router on host: top-2 of softmax(h @ rw.T)."""
    logits = h_flat @ router_weight.T                     # fp32 [T, E]
    lg64 = logits.astype(np.float64)
    p = np.exp(lg64 - lg64.max(axis=1, keepdims=True))
    probs = (p / p.sum(axis=1, keepdims=True)).astype(np.float32)
    # selection by logits order == softmax order (monotonic); ties -> lower idx
    top2 = np.argsort(-logits, axis=1, kind="stable")[:, :TOP_K]
    return top2, probs


def _pack_weights(ws_e, w2s_e):
    wg = ws_e[:I].reshape(IB, 128, DC, 128).transpose(0, 3, 2, 1)
    wu = ws_e[I:].reshape(IB, 128, DC, 128).transpose(0, 3, 2, 1)
    w2 = w2s_e.reshape(DB, 128, IB, 128).transpose(0, 3, 2, 1)
    bf = ml_dtypes.bfloat16
    return (np.ascontiguousarray(wg).astype(bf),
            np.ascontiguousarray(wu).astype(bf),
            np.ascontiguousarray(w2).astype(bf))


def kernel(hidden_states, router_weight, ws, w2s):
    hidden_states = np.asarray(hidden_states, dtype=np.float32)
    router_weight = np.asarray(router_weight, dtype=np.float32)
    ws = np.asarray(ws, dtype=np.float32)
    w2s = np.asarray(w2s, dtype=np.float32)

    b, s, d = hidden_states.shape
    h = hidden_states.reshape(-1, d)
    T = h.shape[0]

    top2, probs = _route_host(h, router_weight)
    # token ids per expert
    idx = [np.nonzero((top2 == e).any(axis=1))[0] for e in range(E)]
    counts = np.array([len(ix) for ix in idx])
    C = max(TN, int(-(-counts.max() // TN)) * TN)

    in_maps = []
    for e in range(E):
        ix = idx[e]
        xe = np.zeros((C, D), np.float32)
        xe[: len(ix)] = h[ix]
        # pack tokens: [C, D] -> [128, DC, C]
        xp = np.ascontiguousarray(
            xe.reshape(C, DC, 128).transpose(2, 1, 0)
        ).astype(ml_dtypes.bfloat16)
        wg, wu, w2 = _pack_weights(ws[e], w2s[e])
        in_maps.append({"x": xp, "wg": wg, "wu": wu, "w2": w2})

    nc = build_kernel(C)
    res = run_bass_kernel_spmd(nc, in_maps, list(range(N_CORES)))

    out = np.zeros((T, D), np.float32)
    for e in range(E):
        ix = idx[e]
        ye = res.results[e]["y"].reshape(D, C).T[: len(ix)]   # [n_e, D]
        w = probs[ix, e][:, None]
        out[ix] += ye * w
    return out.reshape(b, s, d)


# revision 6
# speedup vs baseline: 25.6519x; 25.6519x over previous
"""JambaMoE Trainium2 kernel: expert-parallel MoE with host-side token dispatch.

Strategy (sharding_hint: expert parallelism):
  - 8 experts, 8 cores: core e owns expert e's weights.
  - Router (tiny: [T,2048]@[2048,8]) + top-2 + softmax run on host during
    input sharding; tokens are gathered per expert, padded to a common
    capacity C, and dispatched to the owning core.
  - Each core runs a SiLU-gated MLP (gate/up [4096,2048], down [2048,4096])
    over its C tokens in bf16 (fp32 PSUM accumulation), all data pre-packed
    host-side into DMA-friendly SBUF layouts (contraction dim on partitions).
  - Host scatter-adds the combine-weighted expert outputs back to [B,S,D].

Device kernel is raw Bass (explicit semaphores): this container's walrus
rejects Tile-generated multi-wait instructions ("Too many sync wait
commands"), so all cross-engine sync uses standalone single-sem waits with
cumulative thresholds.
"""

import numpy as np
import ml_dtypes

import concourse.bass as bass
import concourse.mybir as mybir
from concourse.bass_utils import run_bass_kernel_spmd

B, S, D, E, I, TOP_K = 2, 4096, 2048, 8, 4096, 2
N_CORES = 8
TN = 512          # default token tile (free dim per matmul); overridden per-run
DC = D // 128     # 16 contraction chunks for gate/up
IB = I // 128     # 32 intermediate blocks
DB = D // 128     # 16 output-dim blocks
BF16 = mybir.dt.bfloat16
FP32 = mybir.dt.float32


def choose_tiling(maxcount: int):
    """Pick (C, TN): C = TC*TN >= maxcount, TN <= 512 mult of 128, min C."""
    best = None
    for tc in range(1, 9):
        tn = -(-maxcount // (tc * 64)) * 64   # multiple of 64, covers maxcount
        if tn > 512 or tn < 64:
            continue
        c = tc * tn
        if best is None or (c, tc) < best:
            best = (c, tc)
    c, tc = best
    return c, c // tc


def build_kernel(C: int, TN: int, reps: int = 1):
    """Raw-Bass SPMD kernel for one expert shard: y = (silu(x@Wg.T)*(x@Wu.T))@W2.T

    Inputs (packed, see kernel()):
      x:  [128, DC, C]   bf16  (x[t, dc*128+dp] at [dp, dc, t])
      wg: [IB, 128, DC, 128] bf16  (packed gate tiles, contraction on partitions)
      wu: [IB, 128, DC, 128] bf16
      w2: [DB, 128, IB, 128] bf16
    Output:
      y:  [DB, 128, C] fp32  (y[t, db*128+dp] at [db, dp, t])
    """
    TC = (C // TN) * reps   # reps>1: re-run all chunks (timing; idempotent)
    TCR = C // TN
    nslot = 3 if C <= 2560 else 2
    NS2 = 6 if C <= 2304 else 4   # separate slot pool for w2 tiles

    nc = bass.Bass()
    x_ext = nc.dram_tensor("x", [128, DC, C], BF16, kind="ExternalInput")
    wg_ext = nc.dram_tensor("wg", [IB, 128, DC, 128], BF16, kind="ExternalInput")
    wu_ext = nc.dram_tensor("wu", [IB, 128, DC, 128], BF16, kind="ExternalInput")
    w2_ext = nc.dram_tensor("w2", [DB, 128, IB, 128], BF16, kind="ExternalInput")
    y_ext = nc.dram_tensor("y", [DB, 128, C], FP32, kind="ExternalOutput")

    NTMP = 4   # silu temp buffers
    NY = 3     # y staging buffers

    with (
        nc.sbuf_tensor([128, DC, C], BF16) as x_sb,
        nc.sbuf_tensor([128, nslot, DC, 128], BF16) as wg_sb,
        nc.sbuf_tensor([128, nslot, DC, 128], BF16) as wu_sb,
        nc.sbuf_tensor([128, NS2, IB, 128], BF16) as w2_sb,
        nc.sbuf_tensor([128, IB, TN], BF16) as a_sb,
        nc.sbuf_tensor([128, NTMP, TN], FP32) as tmp_sb,
        nc.sbuf_tensor([128, NY, TN], FP32) as y_sb,
        nc.psum_tensor([128, 2, 512], FP32) as g_ps_full,
        nc.psum_tensor([128, 2, 512], FP32) as u_ps_full,
        nc.psum_tensor([128, 4, 512], FP32) as y_ps_full,
        nc.semaphore() as dma_in,   # sync-engine input DMAs (inc 16 each)
        nc.semaphore() as dma_in2,  # scalar-engine input DMAs (x, w2)
        nc.semaphore() as pe_a,     # PE group completions (inc 1)
        nc.semaphore() as act_s,    # ACT silu completions
        nc.semaphore() as dve_s,    # DVE mul/copy completions
        nc.semaphore() as out_s,    # gpsimd output DMAs (inc 16)
        nc.Block() as block,
    ):
        g_ps = g_ps_full[:, :, :TN]   # bank-aligned slots, TN-wide views
        u_ps = u_ps_full[:, :, :TN]
        y_ps = y_ps_full[:, :, :TN]

        # ---- emit-time bookkeeping (python ints; programs are fully static)
        # DMA issue order determines cumulative sem thresholds (per engine).
        # sync engine: wg/wu interleaved per (tc, ib); scalar: x then w2.
        w_ready = {}     # ("g"|"u", tc, ib) -> dma_in thr; ("2", tc, db) -> dma_in2 thr
        for tc in range(TC):
            for ib in range(IB):
                w_ready[("g", tc, ib)] = (tc * 2 * IB + 2 * ib + 1) * 16
                w_ready[("u", tc, ib)] = (tc * 2 * IB + 2 * ib + 2) * 16
        for tc in range(TC):
            for db in range(DB):
                w_ready[("2", tc, db)] = 16 * DC + (tc * DB + db + 1) * 16
        # PE program order: per tc: [g(ib), u(ib)] * IB, then y(db) * DB
        pe_count = 0
        g_end, u_end, y_end = {}, {}, {}
        for tc in range(TC):
            for ib in range(IB):
                pe_count += 1; g_end[(tc, ib)] = pe_count
                pe_count += 1; u_end[(tc, ib)] = pe_count
            for db in range(DB):
                pe_count += 1; y_end[(tc, db)] = pe_count
        # ACT order: silu per (tc, ib)
        silu_end = {}
        cnt = 0
        for tc in range(TC):
            for ib in range(IB):
                cnt += 1; silu_end[(tc, ib)] = cnt
        # DVE order: per tc: mul(ib)*IB then ycopy(db)*DB
        mul_end, ycopy_end = {}, {}
        cnt = 0
        for tc in range(TC):
            for ib in range(IB):
                cnt += 1; mul_end[(tc, ib)] = cnt
            for db in range(DB):
                cnt += 1; ycopy_end[(tc, db)] = cnt
        # output store order on gpsimd
        store_end = {}
        cnt = 0
        for tc in range(TC):
            for db in range(DB):
                cnt += 16; store_end[(tc, db)] = cnt

        @block.sync
        def _(sync):
            # wg/wu weight stream only (x and w2 ride the scalar-engine queue)
            slot_free = {}
            for tc in range(TC):
                for ib in range(IB):
                    s = ib % nslot
                    for kind, sb, ext in (("g", wg_sb, wg_ext), ("u", wu_sb, wu_ext)):
                        key = (kind, s)
                        if key in slot_free:
                            sync.wait_ge(pe_a, slot_free[key])
                        sync.dma_start(sb[:, s], ext[ib]).then_inc(dma_in, 16)
                        slot_free[key] = (g_end if kind == "g" else u_end)[(tc, ib)]

        @block.tensor
        def _(tensor):
            first = True
            for tc in range(TC):
                t0 = (tc % TCR) * TN
                for ib in range(IB):
                    s = ib % nslot
                    gb, ub = ib % 2, ib % 2
                    tensor.wait_ge(dma_in, w_ready[("u", tc, ib)])
                    if first:
                        first = False
                        tensor.wait_ge(dma_in2, 16 * DC)  # x resident
                    # psum bank WAR: consumers of (tc, ib-2) done?
                    if (tc, ib - 2) in mul_end:
                        tensor.wait_ge(dve_s, mul_end[(tc, ib - 2)])
                    elif tc > 0 and ib < 2:
                        tensor.wait_ge(dve_s, mul_end[(tc - 1, IB - 2 + ib)])
                    for dc in range(DC):
                        mm = tensor.matmul(
                            g_ps[:, gb, :], wg_sb[:, s, dc, :],
                            x_sb[:, dc, t0:t0 + TN],
                            start=(dc == 0), stop=(dc == DC - 1),
                        )
                    mm.then_inc(pe_a, 1)
                    for dc in range(DC):
                        mm = tensor.matmul(
                            u_ps[:, ub, :], wu_sb[:, s, dc, :],
                            x_sb[:, dc, t0:t0 + TN],
                            start=(dc == 0), stop=(dc == DC - 1),
                        )
                    mm.then_inc(pe_a, 1)
                for db in range(DB):
                    s = db % NS2
                    yb = db % 4
                    tensor.wait_ge(dma_in2, w_ready[("2", tc, db)])
                    # need all 32 muls of this tc; plus y bank free (copy db-4)
                    need = mul_end[(tc, IB - 1)]
                    if (tc, db - 4) in ycopy_end:
                        need = max(need, ycopy_end[(tc, db - 4)])
                    elif tc > 0:
                        need = max(need, ycopy_end[(tc - 1, DB - 4 + db)])
                    tensor.wait_ge(dve_s, need)
                    for ic in range(IB):
                        mm = tensor.matmul(
                            y_ps[:, yb, :], w2_sb[:, s, ic, :], a_sb[:, ic, :],
                            start=(ic == 0), stop=(ic == IB - 1),
                        )
                    mm.then_inc(pe_a, 1)

        @block.scalar
        def _(scalar):
            for dc in range(DC):
                scalar.dma_start(x_sb[:, dc, :], x_ext[:, dc, :]).then_inc(dma_in2, 16)
            slot2_free = {}

            def w2_load(scalar, tc, db):
                s = db % NS2
                if s in slot2_free:
                    scalar.wait_ge(pe_a, slot2_free[s])
                scalar.dma_start(w2_sb[:, s], w2_ext[db]).then_inc(dma_in2, 16)
                slot2_free[s] = y_end[(tc, db)]

            for tc in range(TC):
                # First NS2 w2 tiles upfront (their slot-free waits reference
                # tc-1 phase B only); the rest must go AFTER this tc's silus:
                # their waits reference this tc's phase B, and a parked wait
                # before a silu would deadlock phase A.
                for db in range(min(NS2, DB)):
                    w2_load(scalar, tc, db)
                for ib in range(IB):
                    ts = ib % NTMP
                    scalar.wait_ge(pe_a, g_end[(tc, ib)])
                    if (tc, ib - NTMP) in mul_end:
                        scalar.wait_ge(dve_s, mul_end[(tc, ib - NTMP)])
                    elif tc > 0 and ib < NTMP:
                        scalar.wait_ge(dve_s, mul_end[(tc - 1, IB - NTMP + ib)])
                    scalar.activation(
                        tmp_sb[:, ts, :], g_ps[:, ib % 2, :],
                        mybir.ActivationFunctionType.Silu,
                    ).then_inc(act_s, 1)
                for db in range(NS2, DB):
                    w2_load(scalar, tc, db)

        @block.vector
        def _(vector):
            for tc in range(TC):
                for ib in range(IB):
                    ts = ib % NTMP
                    vector.wait_ge(act_s, silu_end[(tc, ib)])
                    vector.wait_ge(pe_a, u_end[(tc, ib)])
                    vector.tensor_mul(
                        a_sb[:, ib, :], tmp_sb[:, ts, :], u_ps[:, ib % 2, :]
                    ).then_inc(dve_s, 1)
                for db in range(DB):
                    ys = db % NY
                    vector.wait_ge(pe_a, y_end[(tc, db)])
                    if (tc, db - NY) in store_end:
                        vector.wait_ge(out_s, store_end[(tc, db - NY)])
                    elif tc > 0 and db < NY:
                        vector.wait_ge(out_s, store_end[(tc - 1, DB - NY + db)])
                    vector.tensor_copy(
                        y_sb[:, ys, :], y_ps[:, db % 4, :]
                    ).then_inc(dve_s, 1)

        @block.gpsimd
        def _(gpsimd):
            for tc in range(TC):
                t0 = (tc % TCR) * TN
                for db in range(DB):
                    ys = db % NY
                    gpsimd.wait_ge(dve_s, ycopy_end[(tc, db)])
                    gpsimd.dma_start(
                        y_ext[db, :, t0:t0 + TN], y_sb[:, ys, :]
                    ).then_inc(out_s, 16)
            gpsimd.wait_ge(out_s, 16 * DB * TC)

    return nc


def _route_host(h_flat, router_weight):
    """Replicate the reference router on host: top-2 of softmax(h @ rw.T)."""
    logits = h_flat @ router_weight.T                     # fp32 [T, E]
    lg64 = logits.astype(np.float64)
    p = np.exp(lg64 - lg64.max(axis=1, keepdims=True))
    probs = (p / p.sum(axis=1, keepdims=True)).astype(np.float32)
    # selection by logits order == softmax order (monotonic); ties -> lower idx
    top2 = np.argsort(-logits, axis=1, kind="stable")[:, :TOP_K]
    return top2, probs


def _pack_weights(ws_e, w2s_e):
    wg = ws_e[:I].reshape(IB, 128, DC, 128).transpose(0, 3, 2, 1)
    wu = ws_e[I:].reshape(IB, 128, DC, 128).transpose(0, 3, 2, 1)
    w2 = w2s_e.reshape(DB, 128, IB, 128).transpose(0, 3, 2, 1)
    bf = ml_dtypes.bfloat16
    return (np.ascontiguousarray(wg).astype(bf),
            np.ascontiguousarray(wu).astype(bf),
            np.ascontiguousarray(w2).astype(bf))


def kernel(hidden_states, router_weight, ws, w2s):
    hidden_states = np.asarray(hidden_states, dtype=np.float32)
    router_weight = np.asarray(router_weight, dtype=np.float32)
    ws = np.asarray(ws, dtype=np.float32)
    w2s = np.asarray(w2s, dtype=np.float32)

    b, s, d = hidden_states.shape
    h = hidden_states.reshape(-1, d)
    T = h.shape[0]

    top2, probs = _route_host(h, router_weight)
    # token ids per expert
    idx = [np.nonzero((top2 == e).any(axis=1))[0] for e in range(E)]
    counts = np.array([len(ix) for ix in idx])
    C, tn = choose_tiling(int(counts.max()))

    in_maps = []
    for e in range(E):
        ix = idx[e]
        xe = np.zeros((C, D), np.float32)
        xe[: len(ix)] = h[ix]
        # pack tokens: [C, D] -> [128, DC, C]
        xp = np.ascontiguousarray(
            xe.reshape(C, DC, 128).transpose(2, 1, 0)
        ).astype(ml_dtypes.bfloat16)
        wg, wu, w2 = _pack_weights(ws[e], w2s[e])
        in_maps.append({"x": xp, "wg": wg, "wu": wu, "w2": w2})

    nc = build_kernel(C, tn)
    res = run_bass_kernel_spmd(nc, in_maps, list(range(N_CORES)))

    out = np.zeros((T, D), np.float32)
    for e in range(E):
        ix = idx[e]
        ye = res.results[e]["y"].reshape(D, C).T[: len(ix)]   # [n_e, D]
        w = probs[ix, e][:, None]
        out[ix] += ye * w
    return out.reshape(b, s, d)
